# revision 1
# baseline (speedup 1.0000x reference)
"""ClusterMoCoKnnBert retrieval-knn kernel for 8 Trainium2 NeuronCores.

Contract: kernel(**inputs) takes the FULL (unsharded) inputs and returns the
FULL output, matching reference.reference(). Internally the feature/label/
cluster queues are sharded along K across the 8 cores (liner_q replicated);
each core computes its local cos_sim, the pos/neg masks (fused into the
matmul via a one-hot mask-matmul), local mask counts and a local pos top-k;
the host gathers the per-core results, all-reduce-mins the counts to get
pos_min/neg_min, and re-reduces the gathered candidates into the final
logits tensor.

Everything is hardcoded for the problem sizes:
  B=32, K=131072, H=768, NUM_LABELS=2, CLUSTER_LABELS=16, T=0.07.
"""

import sys

for _p in ("/opt/trn_rl_repo",):
    if _p not in sys.path:
        sys.path.insert(0, _p)

import numpy as np
import ml_dtypes

import concourse.bass as bass
import concourse.bacc as bacc
import concourse.tile as tile
from concourse import mybir
from concourse.bass_utils import run_bass_kernel_spmd

# ---------------------------------------------------------------- constants
B = 32          # batch (queries)
H = 768         # hidden
K = 131072      # queue length
NCORES = 8
KC = K // NCORES          # 16384 local queue columns per core
T = 0.07                  # MoCo temperature
S = 128.0                 # mask shift: masked-out entries get -S (pow2, exact)
NT = 512                  # matmul moving free-dim tile (== one PSUM bank of f32)
STRIPS = 4                # batch strips stacked on partitions (4*32 = 128)
KT = H // 128             # 6 contraction tiles
NLAB = 2
NCLU = 16
NCODE = NLAB * NCLU       # 32 (cluster, label) codes
SENTINEL_CUT = -50.0      # real values are in [-1/T, 1/T] ~ [-14.3, 14.3];
                          # masked entries are <= -S + 1/T < -113

F32 = mybir.dt.float32
F32R = mybir.dt.float32r
BF16 = mybir.dt.bfloat16

# bf16 feature-queue/query mode: halves the dominant DMA traffic (50MB -> 25MB
# per core). Measured on HW: 1.84e-3 absmax-scaled output error (vs 1.2e-4
# for the f32r path) and ~1.7x faster; the kernel is DMA-bound either way.
FQ_BF16 = True


PAIR_OVERRIDE = None   # experiment knob: force groups-per-fetch
TAIL_OVERLAP = True    # run prefix top-k under the last pair's DMA/PE phase
NEG_BF16 = False       # bf16 neg store saves 1MB/core but was part of a
                       # measured regression; f32 keeps neg values exact
LQ_COMPACT = True      # ship [768,32] queries; zero-fill + replicate the
                       # per-strip weight blocks on device (saves ~590KB DMA)
E_SPLIT = False        # streaming eT per group added 32 small DMAs/iter;
                       # real per-DMA overhead cost ~30us -- keep one load
ET_FP8 = False         # fp8e4 one-hot eT is exact and saves 0.5MB DMA, but
                       # measured SLOWER on HW (124.0us vs 115.4us): the fp8
                       # gamma matmuls interleave with bf16 alpha matmuls and
                       # the PE dtype switches cost more than the bytes saved

STRIP_MODE = "blockw"  # "blockw": zero-padded [128,128] block weights per
                       # strip (no tile_position; f32r/bf16 both legal).
                       # "tilepos": PE tile composition (ISA-invalid for f32r
                       # on TRN2 walrus; kept for experiments).


def build_nc(kc: int = KC, fq_bf16: bool | None = None, reps: int = 1,
             strip_mode: str | None = None) -> bass.Bass:
    """Build the single-core Bass program (run SPMD on all 8 cores).

    DRAM interface (per core):
      in  fqT  [H, kc]  f32(r) : transposed local feature_queue chunk
      in  lqT  [H, B]   f32(r) : liner_q.T / T, replicated
      in  eT   [32, kc] bf16   : one-hot of code=cluster*2+label per column
      in  w3T  [32, B]  bf16   : -S * pos_mask(b, code) weight matrix
      out neg  [128, kc/32] f32: cos/T + (-S * pos_mask)  (strip-packed)
      out t16  [128, 16] f32   : per-strip top-16 of pos candidates
      out cnt  [128, groups]f32: per-(strip,group) sum of gamma = -S*pos_count
    """
    groups = kc // (NT * STRIPS)
    assert kc % (NT * STRIPS) == 0
    if fq_bf16 is None:
        fq_bf16 = FQ_BF16
    fq_dt = BF16 if fq_bf16 else F32R
    strip_mode = strip_mode or STRIP_MODE
    blockw = strip_mode == "blockw"
    assert blockw, "tilepos strip mode was removed (ISA-invalid for f32r)"
    WCOL = STRIPS * 128  # zero-padded per-strip weight blocks

    # Bacc (not raw Bass): its compile pipeline splits multi-sem waits
    # (move_matmul_waits_to_ldweights / generate_event_semaphores) to satisfy
    # the TRN2 one-wait-per-instruction constraint walrus enforces.
    nc = bacc.Bacc()
    mask_dt = mybir.dt.float8e4 if ET_FP8 else BF16
    neg_dt = BF16 if NEG_BF16 else F32
    lq_cols = B if LQ_COMPACT else WCOL
    fqT = nc.declare_dram_parameter("fqT", [H, kc], fq_dt, isOutput=False)
    lqT = nc.declare_dram_parameter("lqT", [H, lq_cols], fq_dt, isOutput=False)
    eT = nc.declare_dram_parameter("eT", [NCODE, kc], mask_dt, isOutput=False)
    w3T = nc.declare_dram_parameter("w3T", [NCODE, WCOL], mask_dt, isOutput=False)
    neg = nc.declare_dram_parameter("neg", [128, kc // STRIPS], neg_dt, isOutput=True)
    t16 = nc.declare_dram_parameter("t16", [128, 16], F32, isOutput=True)
    cnt = nc.declare_dram_parameter("cnt", [128, groups], F32, isOutput=True)

    with tile.TileContext(nc) as tc:
        with (
            tc.tile_pool(name="singles", bufs=1) as singles,
            tc.tile_pool(name="fqp", bufs=4) as fqp,
            tc.tile_pool(name="negp", bufs=3) as negp,
            tc.tile_pool(name="gsb", bufs=3) as gsbp,
            tc.tile_pool(name="psum", bufs=2, space="PSUM") as psump,
        ):
            # --- one-time loads -------------------------------------------
            lq_sb = singles.tile([128, KT, WCOL], fq_dt)
            lq_src = lqT[:, :].rearrange("(t p) m -> p t m", p=128)
            if LQ_COMPACT:
                # zero-fill the per-strip weight blocks on device and DMA the
                # compact [H, B] queries into each strip's 32-column window
                nc.gpsimd.memset(lq_sb, 0.0)
                lq4 = lq_sb.rearrange("p t (s c) -> p t s c", s=STRIPS)
                for s in range(STRIPS):
                    nc.sync.dma_start(
                        out=lq4[:, :, s, 32 * s : 32 * s + B], in_=lq_src
                    )
            else:
                nc.sync.dma_start(out=lq_sb, in_=lq_src)
            w3_sb = singles.tile([NCODE, WCOL], mask_dt)
            nc.sync.dma_start(out=w3_sb, in_=w3T[:, :])
            if not E_SPLIT:
                e_sb = singles.tile([NCODE, kc], mask_dt)
                nc.sync.dma_start(out=e_sb, in_=eT[:, :])

            cnt_sb = singles.tile([128, groups], F32)
            pos_cand = singles.tile([128, kc // STRIPS], F32)

            PAIR = PAIR_OVERRIDE or (2 if groups % 2 == 0 else 1)

            def dve_stage(g, alpha, gamma):
                # DVE may read only ONE operand from PSUM per op: stage
                # gamma into SBUF on the (idle) scalar engine, fusing the
                # count reduction into the same instruction's accum_out:
                # sum(gamma) = -S * (#pos in this strip-row group), exact.
                gamma_sb = gsbp.tile([128, NT], F32, tag="gsb")
                nc.scalar.activation(
                    out=gamma_sb,
                    in_=gamma,
                    func=mybir.ActivationFunctionType.Copy,
                    accum_out=cnt_sb[:, g : g + 1],
                )
                # neg sample values: kept entries == cos/T (bf16-rounded on
                # store when NEG_BF16), masked entries <= -S + 1/T
                neg_sb = negp.tile([128, NT], neg_dt, tag="negsb")
                nc.vector.tensor_add(neg_sb, alpha, gamma_sb)
                nc.sync.dma_start(out=neg[:, g * NT : (g + 1) * NT], in_=neg_sb)
                # pos candidates: (alpha - S) - gamma -> kept pos entries
                # are cos/T (+- 1ulp@S), masked entries ~ cos/T - S
                nc.vector.scalar_tensor_tensor(
                    out=pos_cand[:, g * NT : (g + 1) * NT],
                    in0=alpha,
                    scalar=S,
                    in1=gamma_sb,
                    op0=mybir.AluOpType.subtract,
                    op1=mybir.AluOpType.subtract,
                )

            def topk16(dst, src, scratch_free=True):
                # top-16 of src into dst[:, 0:16] (max8 / match_replace /
                # max8); match_replace clobbers src
                nc.vector.max(out=dst[:, 0:8], in_=src)
                nc.vector.match_replace(
                    out=src, in_to_replace=dst[:, 0:8], in_values=src,
                    imm_value=-1e9,
                )
                nc.vector.max(out=dst[:, 8:16], in_=src)

            def body():
                npair = groups // PAIR
                # 1 group == 4 batch-strips x 512 queue columns; PAIR groups
                # share each strip fetch (longer DMA lines, one weight load
                # feeding PAIR matmuls)
                for g2 in range(npair):
                    g0 = g2 * PAIR
                    if TAIL_OVERLAP and g2 == npair - 1 and npair > 1:
                        # top-16 of all finished groups now, emitted before
                        # the last pair's DVE stages so the DVE crunches it
                        # under the final DMA/PE phase instead of as a tail
                        t16a = singles.tile([128, 16], F32)
                        topk16(t16a, pos_cand[:, : g0 * NT])
                    alphas = [
                        psump.tile([128, NT], F32, tag=f"alpha{j}",
                                   name=f"alpha{j}")
                        for j in range(PAIR)
                    ]
                    gammas = [
                        psump.tile([128, NT], F32, tag=f"gamma{j}",
                                   name=f"gamma{j}")
                        for j in range(PAIR)
                    ]
                    for s in range(STRIPS):
                        ncol = (s * groups + g0) * NT
                        fq_t = fqp.tile([128, KT, PAIR * NT], fq_dt, tag="fqt")
                        nc.sync.dma_start(
                            out=fq_t,
                            in_=fqT[:, ncol : ncol + PAIR * NT].rearrange(
                                "(t p) n -> p t n", p=128
                            ),
                        )
                        if E_SPLIT:
                            e_t = fqp.tile([NCODE, PAIR * NT], mask_dt,
                                           tag="et")
                            nc.sync.dma_start(
                                out=e_t, in_=eT[:, ncol : ncol + PAIR * NT]
                            )
                            e_view, e_off = e_t, 0
                        else:
                            e_view, e_off = e_sb, ncol
                        # strip s's [128,128] weight block has the 32 query
                        # columns at partition rows 32s..32s+31 and zeros
                        # elsewhere: all 4 strips accumulate into the full
                        # 128-partition PSUM bank, each contributing exact
                        # +0.0 outside its rows.
                        for kt in range(KT):
                            for j in range(PAIR):
                                nc.tensor.matmul(
                                    alphas[j],
                                    lhsT=lq_sb[:, kt, 128 * s : 128 * (s + 1)],
                                    rhs=fq_t[:, kt, j * NT : (j + 1) * NT],
                                    start=(s == 0 and kt == 0),
                                    stop=(s == STRIPS - 1 and kt == KT - 1),
                                )
                        for j in range(PAIR):
                            nc.tensor.matmul(
                                gammas[j],
                                lhsT=w3_sb[:, 128 * s : 128 * (s + 1)],
                                rhs=e_view[
                                    :, e_off + j * NT : e_off + (j + 1) * NT
                                ],
                                start=(s == 0),
                                stop=(s == STRIPS - 1),
                            )
                    for j in range(PAIR):
                        dve_stage(g0 + j, alphas[j], gammas[j])

                # --- local pos top-16 per strip-row -----------------------
                t16_sb = singles.tile([128, 16], F32)
                if TAIL_OVERLAP and npair > 1:
                    # short tail: top-16 of the last pair's groups, then
                    # merge with the prefix top-16 from inside the loop
                    t16b = singles.tile([128, 16], F32)
                    topk16(t16b, pos_cand[:, (groups - PAIR) * NT :])
                    m32 = singles.tile([128, 32], F32)
                    nc.vector.tensor_copy(m32[:, 0:16], t16a)
                    nc.vector.tensor_copy(m32[:, 16:32], t16b)
                    topk16(t16_sb, m32)
                else:
                    topk16(t16_sb, pos_cand)
                nc.sync.dma_start(out=t16[:, :], in_=t16_sb)
                nc.sync.dma_start(out=cnt[:, :], in_=cnt_sb)

            if reps == 1:
                body()
            else:
                # timing mode: repeat the whole kernel body inside one NEFF
                # so wall-clock deltas measure pure HW execution time
                with tc.For_i(0, reps, 1):
                    body()

    # run the Bacc compile pipeline (register allocation, matmul-wait
    # splitting, event semaphores) before serialization for walrus
    nc.finalize()
    return nc


_NC_CACHE: dict = {}


def _get_nc(kc: int, fq_bf16: bool | None = None, reps: int = 1) -> bass.Bass:
    if fq_bf16 is None:
        fq_bf16 = FQ_BF16
    key = (kc, fq_bf16, reps)
    if key not in _NC_CACHE:
        _NC_CACHE[key] = build_nc(kc, fq_bf16, reps)
    return _NC_CACHE[key]


def make_in_maps(liner_q, feature_queue, label_q, cluster_q, label_queue,
                 cluster_queue, kc: int = KC, ncores: int = NCORES,
                 fq_bf16: bool | None = None):
    """Shard + marshal the full inputs into per-core DRAM input dicts."""
    liner_q = np.asarray(liner_q, dtype=np.float32)
    feature_queue = np.asarray(feature_queue, dtype=np.float32)
    label_q = np.asarray(label_q).astype(np.int64)
    cluster_q = np.asarray(cluster_q).astype(np.int64)
    label_queue = np.asarray(label_queue).astype(np.int64)
    cluster_queue = np.asarray(cluster_queue).astype(np.int64)

    if fq_bf16 is None:
        fq_bf16 = FQ_BF16
    fq_np = ml_dtypes.bfloat16 if fq_bf16 else np.float32
    lqT = np.ascontiguousarray((liner_q / np.float32(T)).T)  # [H, B] f32

    # one-hot code per queue column, bf16 (0/1 exact)
    code = (cluster_queue * NLAB + label_queue).astype(np.int64)  # [K]
    # pos_mask(b, j) for code j=(c*2+l): (c==cluster_q[b]) == (l==label_q[b])
    j = np.arange(NCODE)
    jc, jl = j // NLAB, j % NLAB
    posm = (jc[None, :] == cluster_q[:, None]) == (
        jl[None, :] == label_q[:, None]
    )  # [B, 32]
    w3T = np.ascontiguousarray((-S * posm.astype(np.float32)).T)  # [32, B]

    if STRIP_MODE == "blockw":
        # zero-padded per-strip weight blocks: columns 32s..32s+31 of strip
        # s's [*, 128] block hold the B=32 real columns. With LQ_COMPACT the
        # lq padding happens on device (memset + strided DMA) instead.
        w3_blk = np.zeros((NCODE, STRIPS, 128), np.float32)
        for s in range(STRIPS):
            w3_blk[:, s, 32 * s : 32 * s + B] = w3T
        w3T = w3_blk.reshape(NCODE, STRIPS * 128)
        if not LQ_COMPACT:
            lq_blk = np.zeros((H, STRIPS, 128), np.float32)
            for s in range(STRIPS):
                lq_blk[:, s, 32 * s : 32 * s + B] = lqT
            lqT = lq_blk.reshape(H, STRIPS * 128)
    mask_np = mybir.dt.np(mybir.dt.float8e4) if ET_FP8 else ml_dtypes.bfloat16
    lqT = lqT.astype(fq_np)
    w3T = w3T.astype(mask_np)

    in_maps = []
    for c in range(ncores):
        sl = slice(c * kc, (c + 1) * kc)
        fqT = np.ascontiguousarray(feature_queue[sl].T).astype(fq_np)  # [H, kc]
        eT = np.ascontiguousarray(
            (code[sl][None, :] == j[:, None]).astype(mask_np)
        )  # [32, kc]
        in_maps.append({"fqT": fqT, "lqT": lqT, "eT": eT, "w3T": w3T})
    return in_maps


def assemble(results, top_k, kc: int = KC, ncores: int = NCORES):
    """Gather per-core outputs and re-reduce into the reference layout."""
    groups = kc // (NT * STRIPS)
    # --- counts: all-reduce min across rows for pos_min/neg_min
    pos_cnt = np.zeros(B, dtype=np.int64)
    for r in results:
        c = np.asarray(r["cnt"], dtype=np.float64)  # [128, groups]
        per_row = -(c.reshape(STRIPS, B, groups).sum(axis=(0, 2)) / S)
        pos_cnt += np.rint(per_row).astype(np.int64)
    neg_cnt = kc * ncores - pos_cnt
    pos_min = int(min(int(pos_cnt.min()), int(top_k)))
    neg_min = int(neg_cnt.min())
    assert pos_min > 0 and neg_min > 0

    # --- neg: unscramble strip packing, merge, sort descending
    neg_full = np.empty((B, kc * ncores), dtype=np.float32)
    for ci, r in enumerate(results):
        arr = np.asarray(r["neg"]).astype(np.float32)  # [128, kc/STRIPS]
        # partition s*32+b, col g*NT+j  <->  local k = (s*groups+g)*NT + j
        arr = arr.reshape(STRIPS, B, groups * NT).transpose(1, 0, 2)
        neg_full[:, ci * kc : (ci + 1) * kc] = arr.reshape(B, kc)
    neg_sorted = np.sort(neg_full, axis=1)[:, ::-1][:, :neg_min]

    # --- pos: merge per-(core,strip) top-16 candidates
    cands = np.concatenate(
        [np.asarray(r["t16"]).reshape(STRIPS, B, 16) for r in results], axis=2
    )  # [STRIPS, B, 16*ncores]
    cands = cands.transpose(1, 0, 2).reshape(B, -1)  # [B, 512]
    cands = np.sort(cands, axis=1)[:, ::-1]
    pos_top = cands[:, :pos_min]  # sentinels < -100 can't reach here

    # --- assemble logits_con (values already divided by T on device)
    out = np.empty((B * pos_min, 1 + neg_min), dtype=np.float32)
    ar = np.arange(neg_min)
    for t in range(pos_min):
        out[t::pos_min, 0] = pos_top[:, t]
        idx = (t * neg_min + ar) // pos_min
        out[t::pos_min, 1:] = neg_sorted[:, idx]
    return out


def kernel(liner_q, feature_queue, label_q, cluster_q, label_queue,
           cluster_queue, top_k, reps=1, **run_kwargs):
    top_k = int(np.asarray(top_k).item())
    nc = _get_nc(KC, FQ_BF16, reps)
    in_maps = make_in_maps(
        liner_q, feature_queue, label_q, cluster_q, label_queue, cluster_queue
    )
    res = run_bass_kernel_spmd(nc, in_maps, core_ids=list(range(NCORES)),
                               **run_kwargs)
    out = assemble(res.results, top_k)
    kernel.last_results = res  # stash for profiling in test harness
    return out



# revision 2
# speedup vs baseline: 1.2800x; 1.2800x over previous
"""ClusterMoCoKnnBert retrieval-knn kernel for 8 Trainium2 NeuronCores.

Contract: kernel(**inputs) takes the FULL (unsharded) inputs and returns the
FULL output, matching reference.reference(). Internally the feature/label/
cluster queues are sharded along K across the 8 cores (liner_q replicated);
each core computes F = cos_sim/T + S*pos_mask in ONE fused PE accumulation
chain (the +S*pos_mask comes from a one-hot mask matmul accumulated into the
same PSUM bank), ships F back as bf16 (neg values are the F entries < 64;
pos entries ride at ~128 and are recovered at f32 precision via an on-device
per-strip top-16), and the host re-reduces: exact integer pos/neg counts from
the label/cluster inputs, a global sort of the neg values, and a merge of the
per-(core,strip) top-16 pos candidates.

DMA strategy (the kernel is HBM-bound): the feature queue is pre-packed on
the host into per-iteration [128, 4*6*1024] contiguous blocks so each
iteration needs exactly ONE 6.3MB fully-contiguous DMA (48KB lines); fq loads
alternate between the sync and scalar HWDGE rings so consecutive transfers
overlap their completion latencies, and all stores ride the gpsimd SWDGE ring
so they never serialize against the loads.

Everything is hardcoded for the problem sizes:
  B=32, K=131072, H=768, NUM_LABELS=2, CLUSTER_LABELS=16, T=0.07.
"""

import sys

for _p in ("/opt/trn_rl_repo",):
    if _p not in sys.path:
        sys.path.insert(0, _p)

import numpy as np
import ml_dtypes

import concourse.bass as bass
import concourse.bacc as bacc
import concourse.tile as tile
from concourse import mybir
from concourse.bass_utils import run_bass_kernel_spmd

# ---------------------------------------------------------------- constants
B = 32          # batch (queries)
H = 768         # hidden
K = 131072      # queue length
NCORES = 8
KC = K // NCORES          # 16384 local queue columns per core
T = 0.07                  # MoCo temperature
S = 128.0                 # mask shift: pos entries get +S (pow2, exact)
NT = 512                  # matmul moving free-dim tile (== one PSUM bank of f32)
STRIPS = 4                # batch strips stacked on partitions (4*32 = 128)
KT = H // 128             # 6 contraction tiles
NLAB = 2
NCLU = 16
NCODE = NLAB * NCLU       # 32 (cluster, label) codes
PAIR = 2                  # groups (PSUM banks) per fetch iteration
GROUPS = KC // (NT * STRIPS)   # 8 column groups of NT per strip
NPAIR = GROUPS // PAIR         # 4 fetch iterations per rep
WCOL = STRIPS * 128       # zero-padded per-strip weight blocks
SENTINEL_CUT = -50.0      # pos_cand: kept pos entries ~ cos/T in [-0.3, 0.3];
                          # masked entries ~ cos/T - S < -127
POS_SPLIT = 64.0          # in the F output, pos entries sit at ~S, neg at ~0

F32 = mybir.dt.float32
F32R = mybir.dt.float32r
BF16 = mybir.dt.bfloat16

# bf16 feature-queue/query mode: halves the dominant DMA traffic (50MB -> 25MB
# per core). Measured on HW: ~1.8e-3 absmax-scaled output error, well under
# the 2e-2 gate; the kernel is DMA-bound either way.
FQ_BF16 = True

TAIL_OVERLAP = True    # run prefix top-k under the last pair's DMA/PE phase


def build_nc(kc: int = KC, fq_bf16: bool | None = None, reps: int = 1) -> bass.Bass:
    """Build the single-core Bass program (run SPMD on all 8 cores).

    DRAM interface (per core):
      in  fqP  [NPAIR, 128, STRIPS, KT, PAIR*NT] bf16 : feature queue packed
               into per-iteration contiguous DMA blocks (partition-major)
      in  lqT  [H, B]   bf16 : liner_q.T / T, replicated
      in  eT   [32, kc] bf16 : one-hot of code=cluster*2+label per column
      in  w3T  [32, WCOL] bf16 : +S * pos_mask(b, code), per-strip blocks
      out neg  [NPAIR, 128, PAIR*NT] bf16 : F = cos/T + S*pos_mask
      out t16  [128, 16] f32 : per-strip top-16 of pos candidates (F - S)
    """
    if fq_bf16 is None:
        fq_bf16 = FQ_BF16
    fq_dt = BF16 if fq_bf16 else F32R
    groups, npair = GROUPS, NPAIR
    assert kc == NPAIR * PAIR * STRIPS * NT

    # Bacc (not raw Bass): its compile pipeline splits multi-sem waits
    # (move_matmul_waits_to_ldweights / generate_event_semaphores) to satisfy
    # the TRN2 one-wait-per-instruction constraint walrus enforces.
    nc = bacc.Bacc()
    fqP = nc.declare_dram_parameter(
        "fqP", [npair, 128, STRIPS, KT, PAIR * NT], fq_dt, isOutput=False)
    lqT = nc.declare_dram_parameter("lqT", [H, B], fq_dt, isOutput=False)
    eT = nc.declare_dram_parameter("eT", [NCODE, kc], BF16, isOutput=False)
    w3T = nc.declare_dram_parameter("w3T", [NCODE, WCOL], BF16, isOutput=False)
    neg = nc.declare_dram_parameter(
        "neg", [npair, 128, PAIR * NT], BF16, isOutput=True)
    t16 = nc.declare_dram_parameter("t16", [128, 16], F32, isOutput=True)

    with tile.TileContext(nc) as tc:
        with (
            tc.tile_pool(name="singles", bufs=1) as singles,
            tc.tile_pool(name="fqp", bufs=2 if fq_bf16 else 1) as fqp,
            tc.tile_pool(name="negp", bufs=2) as negp,
            tc.tile_pool(name="psum", bufs=4, space="PSUM") as psump,
        ):
            # --- one-time loads -------------------------------------------
            lq_sb = singles.tile([128, KT, WCOL], fq_dt)
            lq_src = lqT[:, :].rearrange("(t p) m -> p t m", p=128)
            # zero-fill the per-strip weight blocks on device and DMA the
            # compact [H, B] queries into each strip's 32-column window
            nc.gpsimd.memset(lq_sb, 0.0)
            lq4 = lq_sb.rearrange("p t (s c) -> p t s c", s=STRIPS)
            for s in range(STRIPS):
                nc.sync.dma_start(
                    out=lq4[:, :, s, 32 * s : 32 * s + B], in_=lq_src
                )
            w3_sb = singles.tile([NCODE, WCOL], BF16)
            nc.sync.dma_start(out=w3_sb, in_=w3T[:, :])
            e_sb = singles.tile([NCODE, kc], BF16)
            nc.sync.dma_start(out=e_sb, in_=eT[:, :])

            pos_cand = singles.tile([128, kc // STRIPS], F32)

            def topk16(dst, src):
                # top-16 of src into dst[:, 0:16] (max8 / match_replace /
                # max8); match_replace clobbers src
                nc.vector.max(out=dst[:, 0:8], in_=src)
                nc.vector.match_replace(
                    out=src, in_to_replace=dst[:, 0:8], in_values=src,
                    imm_value=-1e9,
                )
                nc.vector.max(out=dst[:, 8:16], in_=src)

            def body():
                # one iteration == 4 batch-strips x PAIR groups of 512 queue
                # columns, fetched as ONE contiguous 6.3MB DMA
                for g2 in range(npair):
                    g0 = g2 * PAIR
                    if TAIL_OVERLAP and g2 == npair - 1 and npair > 1:
                        # top-16 of all finished groups now, emitted before
                        # the last pair's copies so the DVE crunches it
                        # under the final DMA/PE phase instead of as a tail
                        t16a = singles.tile([128, 16], F32)
                        topk16(t16a, pos_cand[:, : g0 * NT])
                    fq_t = fqp.tile([128, STRIPS, KT, PAIR * NT], fq_dt,
                                    tag="fqt")
                    # alternate HWDGE rings (sync / scalar) so consecutive
                    # fetches overlap their fixed completion latencies
                    eng = nc.sync if g2 % 2 == 0 else nc.scalar
                    eng.dma_start(out=fq_t, in_=fqP[g2])
                    alphas = [
                        psump.tile([128, NT], F32, tag=f"alpha{j}",
                                   name=f"alpha{j}")
                        for j in range(PAIR)
                    ]
                    for s in range(STRIPS):
                        ncol = (s * groups + g0) * NT
                        # strip s's [128,128] lq block has the 32 query
                        # columns at partition rows 32s..32s+31 and zeros
                        # elsewhere: all 4 strips accumulate into the full
                        # 128-partition PSUM bank, each contributing exact
                        # +0.0 outside its rows. The mask matmul (one-hot
                        # codes against +S*pos_mask weights) accumulates
                        # into the SAME bank: F = cos/T + S*pos_mask.
                        for kt in range(KT):
                            for j in range(PAIR):
                                nc.tensor.matmul(
                                    alphas[j],
                                    lhsT=lq_sb[:, kt, 128 * s : 128 * (s + 1)],
                                    rhs=fq_t[:, s, kt, j * NT : (j + 1) * NT],
                                    start=(s == 0 and kt == 0),
                                    stop=False,
                                )
                        for j in range(PAIR):
                            nc.tensor.matmul(
                                alphas[j],
                                lhsT=w3_sb[:, 128 * s : 128 * (s + 1)],
                                rhs=e_sb[:, ncol + j * NT : ncol + (j + 1) * NT],
                                start=False,
                                stop=(s == STRIPS - 1),
                            )
                    # stage F to bf16 for the neg output (pos entries ride at
                    # ~128 and only need to sort above the neg band; their
                    # values are recovered at f32 precision via t16)
                    neg_sb = negp.tile([128, PAIR * NT], BF16, tag="negsb")
                    for j in range(PAIR):
                        nc.vector.tensor_copy(
                            neg_sb[:, j * NT : (j + 1) * NT], alphas[j]
                        )
                        # pos candidates: F - S -> kept pos entries are cos/T
                        # (+- 1ulp@S), masked entries ~ cos/T - S
                        nc.scalar.activation(
                            out=pos_cand[:, (g0 + j) * NT : (g0 + j + 1) * NT],
                            in_=alphas[j],
                            func=mybir.ActivationFunctionType.Copy,
                            bias=-S,
                        )
                    nc.gpsimd.dma_start(out=neg[g2], in_=neg_sb)

                # --- local pos top-16 per strip-row -----------------------
                t16_sb = singles.tile([128, 16], F32)
                if TAIL_OVERLAP and npair > 1:
                    # short tail: top-16 of the last pair's groups, then
                    # merge with the prefix top-16 from inside the loop
                    t16b = singles.tile([128, 16], F32)
                    topk16(t16b, pos_cand[:, (groups - PAIR) * NT :])
                    m32 = singles.tile([128, 32], F32)
                    nc.vector.tensor_copy(m32[:, 0:16], t16a)
                    nc.vector.tensor_copy(m32[:, 16:32], t16b)
                    topk16(t16_sb, m32)
                else:
                    topk16(t16_sb, pos_cand)
                nc.gpsimd.dma_start(out=t16[:, :], in_=t16_sb)

            if reps == 1:
                body()
            else:
                # timing mode: repeat the whole kernel body inside one NEFF
                # so wall-clock deltas measure pure HW execution time
                with tc.For_i(0, reps, 1):
                    body()

    # run the Bacc compile pipeline (register allocation, matmul-wait
    # splitting, event semaphores) before serialization for walrus
    nc.finalize()
    return nc


_NC_CACHE: dict = {}


def _get_nc(kc: int, fq_bf16: bool | None = None, reps: int = 1) -> bass.Bass:
    if fq_bf16 is None:
        fq_bf16 = FQ_BF16
    key = (kc, fq_bf16, reps)
    if key not in _NC_CACHE:
        _NC_CACHE[key] = build_nc(kc, fq_bf16, reps)
    return _NC_CACHE[key]


def make_in_maps(liner_q, feature_queue, label_q, cluster_q, label_queue,
                 cluster_queue, kc: int = KC, ncores: int = NCORES,
                 fq_bf16: bool | None = None):
    """Shard + marshal the full inputs into per-core DRAM input dicts."""
    liner_q = np.asarray(liner_q, dtype=np.float32)
    feature_queue = np.asarray(feature_queue, dtype=np.float32)
    label_q = np.asarray(label_q).astype(np.int64)
    cluster_q = np.asarray(cluster_q).astype(np.int64)
    label_queue = np.asarray(label_queue).astype(np.int64)
    cluster_queue = np.asarray(cluster_queue).astype(np.int64)

    if fq_bf16 is None:
        fq_bf16 = FQ_BF16
    fq_np = ml_dtypes.bfloat16 if fq_bf16 else np.float32
    lqT = np.ascontiguousarray((liner_q / np.float32(T)).T)  # [H, B] f32

    # one-hot code per queue column, bf16 (0/1 exact)
    code = (cluster_queue * NLAB + label_queue).astype(np.int64)  # [K]
    # pos_mask(b, j) for code j=(c*2+l): (c==cluster_q[b]) == (l==label_q[b])
    j = np.arange(NCODE)
    jc, jl = j // NLAB, j % NLAB
    posm = (jc[None, :] == cluster_q[:, None]) == (
        jl[None, :] == label_q[:, None]
    )  # [B, 32]
    w3T = np.ascontiguousarray((S * posm.astype(np.float32)).T)  # [32, B]

    # zero-padded per-strip weight blocks: columns 32s..32s+31 of strip
    # s's [*, 128] block hold the B=32 real columns (the lq padding happens
    # on device via memset + strided DMA)
    w3_blk = np.zeros((NCODE, STRIPS, 128), np.float32)
    for s in range(STRIPS):
        w3_blk[:, s, 32 * s : 32 * s + B] = w3T
    w3T = w3_blk.reshape(NCODE, STRIPS * 128)

    lqT = lqT.astype(fq_np)
    w3T = w3T.astype(ml_dtypes.bfloat16)

    in_maps = []
    for c in range(ncores):
        sl = slice(c * kc, (c + 1) * kc)
        fq_local = feature_queue[sl]                 # [kc, H] f32
        # pack into per-iteration contiguous DMA blocks:
        # fqP[g2, p, s, t, n] = fq_local[(s*GROUPS + g2*PAIR)*NT + n, t*128+p]
        X = fq_local.reshape(STRIPS, NPAIR, PAIR * NT, KT, 128)
        fqP = np.ascontiguousarray(
            X.transpose(1, 4, 0, 3, 2)
        ).astype(fq_np)                              # [NPAIR,128,4,KT,1024]
        eTc = np.ascontiguousarray(
            (code[sl][None, :] == j[:, None]).astype(ml_dtypes.bfloat16)
        )  # [32, kc]
        in_maps.append({"fqP": fqP, "lqT": lqT, "eT": eTc, "w3T": w3T})
    return in_maps


def host_counts(label_q, cluster_q, label_queue, cluster_queue):
    """Exact integer pos/neg counts per query row from the label inputs."""
    label_q = np.asarray(label_q).astype(np.int64)
    cluster_q = np.asarray(cluster_q).astype(np.int64)
    code = (np.asarray(cluster_queue).astype(np.int64) * NLAB
            + np.asarray(label_queue).astype(np.int64))
    hist = np.bincount(code, minlength=NCODE)        # [32]
    j = np.arange(NCODE)
    jc, jl = j // NLAB, j % NLAB
    posm = (jc[None, :] == cluster_q[:, None]) == (
        jl[None, :] == label_q[:, None]
    )  # [B, 32]
    pos_cnt = posm @ hist                            # [B]
    neg_cnt = K - pos_cnt
    return pos_cnt, neg_cnt


def assemble(results, top_k, pos_cnt, neg_cnt, kc: int = KC,
             ncores: int = NCORES):
    """Gather per-core outputs and re-reduce into the reference layout."""
    pos_min = int(min(int(pos_cnt.min()), int(top_k)))
    neg_min = int(neg_cnt.min())
    assert pos_min > 0 and neg_min > 0

    # --- neg: unscramble packing, drop the +S pos entries, sort descending
    neg_full = np.empty((B, kc * ncores), dtype=np.float32)
    for ci, r in enumerate(results):
        arr = np.asarray(r["neg"]).astype(np.float32)
        # [g2, s*32+b, j*NT+n]  <->  local k = (s*GROUPS + g2*PAIR + j)*NT + n
        arr = arr.reshape(NPAIR, STRIPS, B, PAIR, NT).transpose(2, 1, 0, 3, 4)
        neg_full[:, ci * kc : (ci + 1) * kc] = arr.reshape(B, kc)
    # pos entries ride at ~S: mask them out by value (neg band is |v| < 1)
    neg_full[neg_full > POS_SPLIT] = -np.inf
    neg_sorted = np.sort(neg_full, axis=1)[:, ::-1][:, :neg_min]

    # --- pos: merge per-(core,strip) top-16 candidates
    cands = np.concatenate(
        [np.asarray(r["t16"]).reshape(STRIPS, B, 16) for r in results], axis=2
    )  # [STRIPS, B, 16*ncores]
    cands = cands.transpose(1, 0, 2).reshape(B, -1)  # [B, 512]
    cands = np.sort(cands, axis=1)[:, ::-1]
    pos_top = cands[:, :pos_min]  # sentinels < -100 can't reach here

    # --- assemble logits_con (values already divided by T on device)
    out = np.empty((B * pos_min, 1 + neg_min), dtype=np.float32)
    ar = np.arange(neg_min)
    for t in range(pos_min):
        out[t::pos_min, 0] = pos_top[:, t]
        idx = (t * neg_min + ar) // pos_min
        out[t::pos_min, 1:] = neg_sorted[:, idx]
    return out


def kernel(liner_q, feature_queue, label_q, cluster_q, label_queue,
           cluster_queue, top_k, reps=1, **run_kwargs):
    top_k = int(np.asarray(top_k).item())
    nc = _get_nc(KC, FQ_BF16, reps)
    in_maps = make_in_maps(
        liner_q, feature_queue, label_q, cluster_q, label_queue, cluster_queue
    )
    res = run_bass_kernel_spmd(nc, in_maps, core_ids=list(range(NCORES)),
                               **run_kwargs)
    pos_cnt, neg_cnt = host_counts(label_q, cluster_q, label_queue,
                                   cluster_queue)
    out = assemble(res.results, top_k, pos_cnt, neg_cnt)
    kernel.last_results = res  # stash for profiling in test harness
    return out
